# revision 29
# baseline (speedup 1.0000x reference)
"""TRN2 Bass kernel for nn_CrossAttentionScorer.

The module collapses algebraically: seq-len is 1, so softmax over the single
attention score is identically 1.0 and the attention output equals `v`
exactly — the whole q/k path is dead code. The remaining computation is

    z   = layernorm(candidate)                 (ln weight folded into W)
    out = relu(z @ W.T + bh) @ sign_vec + b2

with W = (|w2| * (w1 @ wo @ wv) * ln_w) folded on the host and sign(w2)
handled by permuting FF columns so the final dot product becomes
sum(relu(pos block)) - sum(relu(neg block)).

bf16 datapath: x and W are cast to bf16 on the host, so weights DMA straight
into matmul-ready SBUF tiles (no on-chip cast — this was the old startup
bottleneck) and x DMA traffic is halved. PSUM accumulation stays fp32; the
LN statistics accumulate in fp32. relu(rsig*(xc@W)) = rsig*relu(xc@W) since
rsig>0, so the variance/rsqrt chain is off the critical path: ScalarE does
the front-end (center), TensorE transposes xc and runs the bf16 matmuls,
VectorE does relu-and-accumulate + the final rsig scale.

Data parallel over 8 NeuronCores: batch 32768 -> 8 x 4096 rows; weights
replicated.
"""

import numpy as np

_B, _D, _FF = 32768, 1024, 2048
_NC = 8
_P = 128
_SHARD = _B // _NC     # 4096 rows per core
_NTILE = _SHARD // _P  # 32 tiles of 128 rows
_KC = _D // _P         # 8 contraction chunks
_NTW = 512             # matmul moving free size (one PSUM bank of fp32)
_NFT = _FF // _NTW     # 4 ff tiles

_program_cache = {}


def _build_program(P0: int, has_bias: bool, ntile: int = _NTILE):
    import concourse.bacc as bacc
    import concourse.mybir as mybir
    import concourse.tile as tile
    from concourse import masks
    from contextlib import ExitStack

    f32 = mybir.dt.float32
    bf16 = mybir.dt.bfloat16
    AF = mybir.ActivationFunctionType
    AX = mybir.AxisListType
    ALU = mybir.AluOpType

    shard = ntile * _P
    nc = bacc.Bacc("TRN2", target_bir_lowering=False, debug=False)
    x_d = nc.dram_tensor("x", [shard, _D], bf16, kind="ExternalInput")
    # weights pre-shuffled on host: blocks indexed [nt][kc] -> [128, 512]
    wt_d = nc.dram_tensor("wt", [_NFT * _KC * _P, _NTW], bf16,
                          kind="ExternalInput")
    if has_bias:
        bh_d = nc.dram_tensor("bh", [1, _FF], f32, kind="ExternalInput")
    o_d = nc.dram_tensor("o", [shard, 1], f32, kind="ExternalOutput")

    inv_d = 1.0 / _D

    # pos/neg split per ff tile (pos block is a prefix after host permutation)
    slices = []  # (nt, lo, hi, sign)
    for nt in range(_NFT):
        lo, hi = nt * _NTW, (nt + 1) * _NTW
        npos = min(max(P0 - lo, 0), _NTW)
        if npos > 0:
            slices.append((nt, 0, npos, 1.0))
        if npos < _NTW:
            slices.append((nt, npos, _NTW, -1.0))
    ncol = len(slices)
    kp = sum(1 for s in slices if s[3] > 0)   # pos cols are a prefix

    with tile.TileContext(nc) as tc, ExitStack() as ctx:
        const = ctx.enter_context(tc.tile_pool(name="const", bufs=1))
        wpool = ctx.enter_context(tc.tile_pool(name="w", bufs=1))
        xpool = ctx.enter_context(tc.tile_pool(name="xp", bufs=6))
        zpool = ctx.enter_context(tc.tile_pool(name="zp", bufs=3))
        ztpool = ctx.enter_context(tc.tile_pool(name="ztp", bufs=10))
        dpool = ctx.enter_context(tc.tile_pool(name="dump", bufs=1))
        spool = ctx.enter_context(tc.tile_pool(name="sp", bufs=24))
        apool = ctx.enter_context(tc.tile_pool(name="acc", bufs=10))
        opool = ctx.enter_context(tc.tile_pool(name="op", bufs=4))
        pst = ctx.enter_context(
            tc.tile_pool(name="pst", bufs=2 if has_bias else 3, space="PSUM"))
        psm = ctx.enter_context(
            tc.tile_pool(name="psm", bufs=3 if has_bias else 4, space="PSUM"))
        psx = ctx.enter_context(tc.tile_pool(name="psx", bufs=1, space="PSUM"))

        ident32 = const.tile([_P, _P], f32)
        masks.make_identity(nc, ident32[:])
        ident = const.tile([_P, _P], bf16)
        nc.vector.tensor_copy(ident[:], ident32[:])
        epsT = const.tile([_P, 1], f32)
        nc.gpsimd.memset(epsT[:], 1e-5)

        # Pre-load ScalarE activation tables (Identity/Square, Sqrt) with
        # dummy ops so the ~1.3us ACT_TABLE_LOADs happen while waiting for
        # the first x tile instead of on tile 0's critical path.
        warmL = const.tile([_P, _P], bf16, tag="warmL")
        nc.gpsimd.memset(warmL[:], 1.0)
        warm = const.tile([_P, _NTW], bf16, tag="warm")
        nc.gpsimd.memset(warm[:], 1.0)
        tdum = dpool.tile([_P, 1], f32, tag="tdum")
        nc.scalar.activation(tdum[:], epsT[:], AF.Square)
        nc.scalar.activation(tdum[:], epsT[:], AF.Sqrt)

        # Pre-warm the PE HAM clock gate: ~4.5us of throwaway matmuls on a
        # memset tile flip the PE to 2.4GHz before the first real matmul
        # group arrives (~3.4us of sustained PE activity required).
        wdum = psx.tile([_P, _NTW], f32, tag="wdum")
        for i in range(10):
            nc.tensor.matmul(wdum[:], warmL[:], warm[:],
                             start=(i == 0), stop=(i == 9))

        # weights: direct bf16 DMA into matmul-ready layout, no casts.
        # wtr columns ordered (nt, kc): chunk at (nt*_KC + kc) * _NTW.
        # Batched: one InstDMACopy covers several kc chunks (3D AP) — small
        # per-chunk DMAs serialize on the HWDGE ring at ~2us each.
        wtr = wpool.tile([_P, _NFT * _KC * _NTW], bf16)
        wt_r = wt_d.rearrange("(nt kc p) n -> nt p kc n", p=_P, kc=_KC)

        def load_weights(nt, eng, nsplit=1):
            step = _KC // nsplit
            for s in range(nsplit):
                k0 = s * step
                col = (nt * _KC + k0) * _NTW
                eng.dma_start(wtr[:, col:col + step * _NTW],
                              wt_r[nt][:, k0:k0 + step])

        if has_bias:
            bh32 = const.tile([1, _FF], f32)
            bhr = const.tile([1, _FF], bf16)
            nc.sync.dma_start(bh32[:], bh_d[:, :])
            nc.vector.tensor_copy(bhr[:], bh32[:])

        x_r = x_d.rearrange("(t p) d -> t p d", p=_P)
        o_r2 = o_d.rearrange("(t p) one -> t (p one)", p=_P)

        # Software-pipelined emission: F(t) = load/stats/center/transpose,
        # B(t) = matmuls/relu-accum/combine/store. Emitting F(t+1) before B(t)
        # keeps each engine's FIFO free of head-of-line blocking.
        state = {}
        xtiles = {}

        x_r2 = x_d.rearrange("(u two p) d -> u p two d", p=_P, two=2)

        def load(t, eng=None):
            x = xpool.tile([_P, _D], bf16)
            (eng or nc.gpsimd).dma_start(x[:], x_r[t])
            xtiles[t] = x

        def load_pair(u):
            xx = xpool.tile([_P, 2 * _D], bf16, tag="xpair")
            nc.gpsimd.dma_start(xx[:], x_r2[u])
            xtiles[2 * u] = xx[:, 0:_D]
            xtiles[2 * u + 1] = xx[:, _D:2 * _D]

        class _APBox:
            def __init__(self, ap):
                self.ap = ap

            def __getitem__(self, k):
                return self.ap

        def front(t):
            x = xtiles.pop(t)[:]

            # VectorE: row sum then scale by -1/D (two short ops beat one
            # fused dual-ALU pass over the full tile)
            xsum = spool.tile([_P, 1], f32)
            nc.vector.reduce_sum(xsum[:], x, axis=AX.X)
            negmu = spool.tile([_P, 1], f32)
            nc.vector.tensor_scalar_mul(negmu[:], xsum[:], -inv_d)
            # ScalarE critical path: center
            xc = zpool.tile([_P, _D], bf16)
            nc.scalar.activation(xc[:], x, AF.Identity, bias=negmu[:])

            # PE transpose xc -> zt; ScalarE does the PSUM->SBUF drain.
            # (A DMA-xbar transpose variant measured 2.7x slower end-to-end:
            # 256 32KB transposes are fixed-cost dominated on the HWDGE ring.)
            zt = ztpool.tile([_P, _D], bf16)
            for half in range(2):
                tp = pst.tile([_P, _NTW], bf16)
                for j in range(4):
                    c = half * 4 + j
                    nc.tensor.transpose(tp[:, j * _P:(j + 1) * _P],
                                        xc[:, c * _P:(c + 1) * _P], ident[:])
                nc.scalar.activation(zt[:, half * _NTW:(half + 1) * _NTW],
                                     tp[:], AF.Identity)

            # variance chain (only gates the final per-row scale; emitted after
            # the transposes so it never delays what TensorE waits on)
            dump2 = dpool.tile([_P, _D], f32, tag="dump2")
            s2 = spool.tile([_P, 1], f32)
            nc.scalar.activation(dump2[:], xc[:], AF.Square, accum_out=s2[:])
            v = spool.tile([_P, 1], f32)
            nc.scalar.activation(v[:], s2[:], AF.Identity, scale=inv_d, bias=epsT[:])
            rv = spool.tile([_P, 1], f32)
            nc.vector.reciprocal(rv[:], v[:])
            rsig = spool.tile([_P, 1], f32)
            nc.scalar.activation(rsig[:], rv[:], AF.Sqrt)

            st = {"zt": zt, "rsig": rsig}
            if has_bias:
                # sqrt(var+eps) = v * rsig; transposed below for the rank-1
                # bias matmul (bh enters pre-relu as (1/rsig)_b * bh_ff)
                sqv = spool.tile([_P, 1], f32)
                nc.vector.tensor_mul(sqv[:], v[:], rsig[:])
                sqvb = spool.tile([_P, 1], bf16, tag="sqvb")
                nc.vector.tensor_copy(sqvb[:], sqv[:])
                tpb = pst.tile([_P, _P], bf16, tag="tpb")
                nc.tensor.transpose(tpb[:], sqvb[:].to_broadcast((_P, _P)), ident[:])
                sqvr = ztpool.tile([1, _P], bf16, tag="sqvr")
                nc.vector.tensor_copy(sqvr[:], tpb[0:1, :])
                st["sqvr"] = sqvr
            state[t] = st

        accs = {}
        cols = {}

        def mm_group(t, nt):
            st = state[t]
            zt = st["zt"]
            if nt == 0:
                acc_t = apool.tile([_P, ncol], f32, tag="acc")
                accs[t] = acc_t
                cols[t] = 0
            acc = accs[t]
            hdump = dpool.tile([_P, _NTW], f32, tag="hdump")
            ps = psm.tile([_P, _NTW], f32)
            for kc in range(_KC):
                nc.tensor.matmul(
                    ps[:], zt[:, kc * _P:(kc + 1) * _P],
                    wtr[:, (nt * _KC + kc) * _NTW:
                           (nt * _KC + kc + 1) * _NTW],
                    start=(kc == 0),
                    stop=(kc == _KC - 1 and not has_bias))
            if has_bias:
                nc.tensor.matmul(ps[:], st["sqvr"][:],
                                 bhr[:, nt * _NTW:(nt + 1) * _NTW],
                                 start=False, stop=True)
            for (snt, lo, hi, sgn) in slices:
                if snt != nt:
                    continue
                nc.vector.tensor_scalar(
                    out=hdump[:, lo:hi], in0=ps[:, lo:hi],
                    scalar1=0.0, scalar2=None,
                    op0=ALU.max, op1=ALU.add,
                    accum_out=acc[:, cols[t]:cols[t] + 1])
                cols[t] += 1

        # per-tile outputs collect as columns of one SBUF tile; a single
        # transpose + one 16KB store at the end replaces 32 tiny stores
        # (whose ~2us completion receipts serialized on the sync ring).
        ocol = opool.tile([_P, ntile], f32, tag="ocol")

        def finish(t):
            st = state.pop(t)
            rsig = st["rsig"]
            acc = accs.pop(t)
            assert cols.pop(t) == ncol
            # combine: out = rsig * (sum(pos cols) - sum(neg cols))
            if 0 < kp < ncol:
                oP = spool.tile([_P, 1], f32, tag="oP")
                oN = spool.tile([_P, 1], f32, tag="oN")
                nc.vector.reduce_sum(oP[:], acc[:, 0:kp], axis=AX.X)
                nc.vector.reduce_sum(oN[:], acc[:, kp:ncol], axis=AX.X)
                S = spool.tile([_P, 1], f32, tag="S")
                nc.vector.tensor_sub(S[:], oP[:], oN[:])
            else:
                S = spool.tile([_P, 1], f32, tag="S")
                nc.vector.reduce_sum(S[:], acc[:, 0:ncol], axis=AX.X)
                if kp == 0:
                    nc.vector.tensor_scalar_mul(S[:], S[:], -1.0)
            nc.vector.tensor_mul(ocol[:, t:t + 1], S[:], rsig[:])

        def store_out():
            po = psx.tile([_P, _NTW], f32, tag="wdum")
            nc.tensor.transpose(po[0:ntile, 0:_P], ocol[:], ident32[:])
            oT = opool.tile([_P, _P], f32, tag="oT")
            nc.scalar.activation(oT[0:ntile, :], po[0:ntile, 0:_P], AF.Identity)
            nc.sync.dma_start(o_r2, oT[0:ntile, :])

        # All weight DMAs go on the sync HWDGE ring, first ff-tile first and
        # split in two so the first matmul group isn't gated on a full 1MB
        # transfer. Output stores follow on the same ring. x loads run on
        # the gpsimd (SWDGE) ring — the rings drain independently, and the
        # scalar ring carries no DMAs at all (a queued DMA would head-of-line
        # block ScalarE's center ops for the ring's duration).
        #
        # Tiles are processed in groups of TG with the ff-tile loop OUTER:
        # group g runs (nt=0: t0..t3), (nt=1: t0..t3), ... so ff-tile nt's
        # weights are not needed until ~nt*TG*1.8us into the matmul stream —
        # the 1MB-per-nt weight DMAs land under 8-core HBM contention without
        # stalling the PE. Fronts for group g+1 are interleaved into group
        # g's nt=1/2 rounds; x loads stay ~4 tiles ahead of fronts.
        _TG = 4
        # the first two x tiles ride the sync HWDGE ring ahead of the weight
        # burst (~0.6us first-byte vs ~1us SWDGE emission queued behind the
        # const-pool memsets) — tile 0's front end is the critical path.
        # Later x tiles arrive as 512KB pairs on the gpsimd ring: half the
        # SWDGE emissions and completion semaphores.
        load(0, nc.sync)
        load(1, nc.sync)
        load_weights(0, nc.sync, nsplit=2)
        for nt in range(1, _NFT):
            load_weights(nt, nc.sync, nsplit=2)
        nextload = 2
        while nextload + 1 < min(_TG + 8, ntile):
            load_pair(nextload // 2)
            nextload += 2
        for t in range(min(_TG, ntile)):
            front(t)
        nextfront = min(_TG, ntile)
        ngroup = (ntile + _TG - 1) // _TG
        for g in range(ngroup):
            base = g * _TG
            for nt in range(_NFT):
                for tt in range(_TG):
                    t = base + tt
                    if t >= ntile:
                        continue
                    if nt == 1 and nextfront < ntile:
                        if nextfront % 2 == 0 and nextload + 1 < ntile:
                            load_pair(nextload // 2)
                            nextload += 2
                        elif nextfront % 2 == 0 and nextload < ntile:
                            load(nextload)
                            nextload += 1
                        front(nextfront)
                        nextfront += 1
                    mm_group(t, nt)
            for tt in range(_TG):
                if base + tt < ntile:
                    finish(base + tt)
        store_out()

    nc.compile()
    return nc


def _get_program(P0: int, has_bias: bool):
    key = (P0, has_bias)
    if key not in _program_cache:
        _program_cache[key] = _build_program(P0, has_bias)
    return _program_cache[key]


def _fold_weights(inputs):
    gd = lambda k: np.asarray(inputs[k], dtype=np.float64)
    wv, wo, w1, w2 = gd("wv"), gd("wo"), gd("w1"), gd("w2")
    bv, bo, b1, b2 = gd("bv"), gd("bo"), gd("b1"), gd("b2")
    lnw, lnb = gd("ln_kv_w"), gd("ln_kv_b")

    M = w1 @ wo @ wv                              # [FF, D]
    bias_h = M @ lnb + w1 @ (wo @ bv + bo) + b1   # [FF]
    We = M * lnw[None, :]                         # fold LN weight into columns

    w2v = w2.reshape(-1)                          # [FF]
    aw2 = np.abs(w2v)
    sgn = np.sign(w2v)
    perm = np.argsort(-sgn, kind="stable")        # +1 block, then 0, then -1
    P0 = int((sgn >= 0).sum())

    Wf = (We * aw2[:, None])[perm]                # [FF, D]
    bf = (bias_h * aw2)[perm]                     # [FF]

    Wt = np.ascontiguousarray(Wf.T)                      # [D, FF] f64
    bh = bf.astype(np.float32)[None, :]                  # [1, FF]
    has_bias = bool(np.any(bh != 0.0))
    return Wt, bh, has_bias, P0, float(b2.reshape(-1)[0])


def kernel(run_opts=None, **inputs):
    """Full inputs in, full [B, 1] float32 output out. 8-core data parallel."""
    import ml_dtypes
    from concourse.bass_utils import run_bass_kernel_spmd

    bf16 = ml_dtypes.bfloat16
    x = np.asarray(inputs["candidate_feature"], dtype=np.float32).astype(bf16)

    Wt, bh, has_bias, P0, b2 = _fold_weights(inputs)
    nc = _get_program(P0, has_bias)

    # shuffle W into per-(nt, kc) [128, 512] blocks, bf16
    Wb = Wt.reshape(_KC, _P, _NFT, _NTW)              # [kc, p, nt, n]
    Wb = np.ascontiguousarray(Wb.transpose(2, 0, 1, 3))  # [nt, kc, p, n]
    Wb = Wb.astype(bf16).reshape(_NFT * _KC * _P, _NTW)

    common = {"wt": Wb}
    if has_bias:
        common["bh"] = bh
    in_maps = []
    for i in range(_NC):
        m = dict(common)
        m["x"] = np.ascontiguousarray(x[i * _SHARD:(i + 1) * _SHARD])
        in_maps.append(m)

    res = run_bass_kernel_spmd(nc, in_maps, core_ids=list(range(_NC)),
                               **(run_opts or {}))
    out = np.concatenate([r["o"] for r in res.results], axis=0)
    if b2 != 0.0:
        out = out + np.float32(b2)
    if run_opts:
        kernel.last_results = res
    return out.astype(np.float32)
